# revision 10
# baseline (speedup 1.0000x reference)
"""ArcMarginProduct (ArcFace) forward on 8 TRN2 NeuronCores.

out[b, c] = s * cos(theta_bc)         except at c == label[b] where
out[b, c] = s * phi(cos(theta_bc))    (margin epilogue)

Strategy (classification-parallel / Partial-FC), v4:
  - pad C 84281 -> 86016 = 8 * 10752, shard class rows across 8 cores;
    Q=4 classes per partition line -> 8KB w-load / 4KB out-store
    descriptors per partition
  - host precomputes xt = bf16((s * x / ||x||).T) and winv = 1/||w_c||;
    device computes out^T[c, b] = (w_bf16 @ xt) * winv[c] via PE
    transposes + matmuls, with the per-class scale folded into the
    PSUM->SBUF eviction
  - margin epilogue (512 scattered elements) applied on host
  - w-load DMAs ride the scalar(Act) HW DGE queue (prefetch depth 2),
    out-store DMAs the sync(SP) queue
  - per-chunk engine split: scalar casts 2 w rows + evicts 2 psum rows,
    vector casts 2 + evicts 2 + does the 4 wT PSUM->SBUF copies
  - host concatenates shards, drops padding, transposes, casts to f32
"""

import math

import numpy as np

B = 512
D = 512
C = 84281
NCORES = 8
Q = 4                  # classes packed per partition line
TILE = 128 * Q         # 512 classes per tile
NT = 21                # tiles per core
CS = NT * TILE         # 10752 padded classes per core
REAL = [10536] * 7 + [C - 10536 * 7]   # real class rows per core
BASE = [10536 * i for i in range(NCORES)]
PF = 2                 # w-DMA prefetch depth in tiles

S_SCALE = 32.0
MARGIN = 0.5
COS_M = math.cos(MARGIN)
SIN_M = math.sin(MARGIN)
TH = math.cos(math.pi - MARGIN)
MM = math.sin(math.pi - MARGIN) * MARGIN

_CACHE = {}


def _build_nc():
    import concourse.tile as tile
    from concourse import bacc, mybir
    from concourse.masks import make_identity
    from contextlib import ExitStack

    f32 = mybir.dt.float32
    bf16 = mybir.dt.bfloat16

    nc = bacc.Bacc("TRN2", target_bir_lowering=False, debug=False, num_devices=NCORES)
    w_ext = nc.declare_dram_parameter("weight", [CS, D], f32, isOutput=False)
    xt_ext = nc.declare_dram_parameter("xt", [D, B], bf16, isOutput=False)
    winv_ext = nc.declare_dram_parameter("winv", [CS], f32, isOutput=False)
    out_ext = nc.declare_dram_parameter("out", [CS, B], bf16, isOutput=True)

    # class g = t*TILE + p*Q + q  ->  partition p, tile t, row q
    w_view = w_ext[:].rearrange("(t p q) d -> p t q d", p=128, q=Q)
    xt_view = xt_ext[:].rearrange("(k p) b -> p k b", p=128)        # [128, 4, B]
    winv_view = winv_ext[:].rearrange("(t p q) -> p t q", p=128, q=Q)
    out_view = out_ext[:].rearrange("(t p q) b -> p t q b", p=128, q=Q)

    with tile.TileContext(nc) as tc, ExitStack() as es:
        cpool = es.enter_context(tc.tile_pool(name="consts", bufs=1))
        wpool = es.enter_context(tc.tile_pool(name="wch", bufs=4))
        nbpool = es.enter_context(tc.tile_pool(name="wnb", bufs=3))
        outpool = es.enter_context(tc.tile_pool(name="outch", bufs=3))
        wtpool = es.enter_context(tc.tile_pool(name="wt", bufs=3))
        ppool_out = es.enter_context(tc.tile_pool(name="pout", bufs=3, space="PSUM"))
        ppool_wt = es.enter_context(tc.tile_pool(name="pwt", bufs=2, space="PSUM"))

        # ---- w prefetch: rows q0-2 on the scalar(Act) HWDGE queue, row q3
        # on the sync(SP) queue -> both queues share the 22MB input stream
        wch_tiles = []

        def issue_w_dma(t):
            wch = wpool.tile([128, Q, D], f32, tag="wch", name="wch")
            nc.scalar.dma_start(out=wch[:, 0:3, :], in_=w_view[:, t, 0:3, :])
            nc.sync.dma_start(out=wch[:, 3, :], in_=w_view[:, t, 3, :])
            wch_tiles.append(wch)

        for t in range(PF):
            issue_w_dma(t)

        ident = cpool.tile([128, 128], f32, tag="ident")
        make_identity(nc, ident[:])
        ident_bf = cpool.tile([128, 128], bf16, tag="ident_bf")
        nc.vector.tensor_copy(ident_bf[:], ident[:])

        # ---- one-shot loads: xt (pre-normalized, pre-scaled, bf16) + winv
        xnT = cpool.tile([128, 4, B], bf16, tag="xnT")
        nc.sync.dma_start(out=xnT[:], in_=xt_view)
        winv_sb = cpool.tile([128, NT, Q], f32, tag="winv_sb")
        nc.sync.dma_start(out=winv_sb[:], in_=winv_view)

        def prep(t):
            """cast w rows to bf16 (scalar q0, vector q1, gpsimd q2/q3)."""
            if t + PF < NT:
                issue_w_dma(t + PF)
            wch = wch_tiles[t]
            wnb = nbpool.tile([128, Q, D], bf16, tag="wnb", name="wnb")
            nc.scalar.activation(
                out=wnb[:, 0, :],
                in_=wch[:, 0, :],
                func=mybir.ActivationFunctionType.Copy,
            )
            nc.vector.tensor_copy(wnb[:, 1, :], wch[:, 1, :])
            for q in range(2, Q):
                nc.gpsimd.tensor_copy(wnb[:, q, :], wch[:, q, :])
            return wnb

        def pe(t, wnb):
            pos = []
            for g0 in (0, 2):
                po = ppool_out.tile([128, 2 * B], f32, name="po")
                for jj in range(2):
                    j = g0 + jj
                    pwt = ppool_wt.tile([128, D], bf16, name="pwt")
                    for k in range(4):
                        nc.tensor.transpose(
                            pwt[:, k * 128 : (k + 1) * 128],
                            wnb[:, j, k * 128 : (k + 1) * 128],
                            ident_bf[:],
                        )
                    wT = wtpool.tile([128, D], bf16, tag="wT", name="wT")
                    nc.vector.tensor_copy(wT[:], pwt[:])
                    for k in range(4):
                        nc.tensor.matmul(
                            po[:, jj * B : (jj + 1) * B],
                            lhsT=wT[:, k * 128 : (k + 1) * 128],
                            rhs=xnT[:, k, :],
                            start=(k == 0),
                            stop=(k == 3),
                        )
                pos.append(po)
            return pos

        def outcopy(t, pos):
            """PSUM -> SBUF eviction with winv[c] fold (scalar q0/q1,
            vector q2/q3), then SP-queue store."""
            outch = outpool.tile([128, Q, B], bf16, tag="outch", name="outch")
            for q in range(Q):
                po = pos[q // 2]
                src = po[:, (q % 2) * B : (q % 2 + 1) * B]
                wv = winv_sb[:, t, q : q + 1]
                if q < 2:
                    nc.scalar.activation(
                        out=outch[:, q, :],
                        in_=src,
                        func=mybir.ActivationFunctionType.Copy,
                        scale=wv,
                    )
                else:
                    nc.vector.tensor_scalar_mul(outch[:, q, :], src, wv)
            nc.sync.dma_start(out=out_view[:, t, :, :], in_=outch[:])

        wnb_prev = prep(0)
        pos_prev = None
        for t in range(NT):
            if pos_prev is not None:
                outcopy(t - 1, pos_prev)
            pos = pe(t, wnb_prev)
            if t + 1 < NT:
                wnb_prev = prep(t + 1)
            pos_prev = pos
        outcopy(NT - 1, pos_prev)

    nc.finalize()
    return nc


def _get_nc():
    if "nc" not in _CACHE:
        _CACHE["nc"] = _build_nc()
    return _CACHE["nc"]


def make_in_maps(x, weight, label):
    import ml_dtypes

    x = np.asarray(x, dtype=np.float32)
    weight = np.asarray(weight, dtype=np.float32)
    xn = x / np.maximum(
        np.linalg.norm(x, axis=1, keepdims=True), 1e-12
    )
    xt = np.ascontiguousarray((S_SCALE * xn).T).astype(ml_dtypes.bfloat16)
    in_maps = []
    for i in range(NCORES):
        a, r = BASE[i], REAL[i]
        wshard = np.ones((CS, D), dtype=np.float32)
        wshard[:r] = weight[a : a + r]
        wn = np.maximum(np.sqrt(np.einsum("cd,cd->c", wshard, wshard)), 1e-12)
        winv = (1.0 / wn).astype(np.float32)
        in_maps.append({"weight": wshard, "xt": xt, "winv": winv})
    return in_maps


def assemble(results, label):
    shards = [np.asarray(results[i]["out"])[: REAL[i]] for i in range(NCORES)]
    full_t = np.concatenate(shards, axis=0).astype(np.float32)  # [C, B]
    out = np.ascontiguousarray(full_t.T)                        # [B, C]
    # margin epilogue on the 512 label positions
    label = np.asarray(label).astype(np.int64)
    b = np.arange(B)
    cosv = out[b, label] / S_SCALE
    sine = np.sqrt(np.maximum(0.0, 1.0 - cosv * cosv))
    phi = cosv * COS_M - sine * SIN_M
    out[b, label] = np.where(cosv - TH > 0, phi, cosv - MM) * S_SCALE
    return out


def kernel(x, weight, label):
    from concourse.bass_utils import run_bass_kernel_spmd

    nc = _get_nc()
    in_maps = make_in_maps(x, weight, label)
    res = run_bass_kernel_spmd(nc, in_maps, list(range(NCORES)))
    return assemble(res.results, label)


# revision 11
# speedup vs baseline: 1.0746x; 1.0746x over previous
"""ArcMarginProduct (ArcFace) forward on 8 TRN2 NeuronCores.

out[b, c] = s * cos(theta_bc)         except at c == label[b] where
out[b, c] = s * phi(cos(theta_bc))    (margin epilogue)

Strategy (classification-parallel / Partial-FC), v4:
  - pad C 84281 -> 86016 = 8 * 10752, shard class rows across 8 cores;
    Q=4 classes per partition line -> 8KB w-load / 4KB out-store
    descriptors per partition
  - host precomputes xt = bf16((s * x / ||x||).T) and winv = 1/||w_c||;
    device computes out^T[c, b] = (w_bf16 @ xt) * winv[c] via PE
    transposes + matmuls, with the per-class scale folded into the
    PSUM->SBUF eviction
  - margin epilogue (512 scattered elements) applied on host
  - w-load DMAs ride the scalar(Act) HW DGE queue (prefetch depth 2),
    out-store DMAs the sync(SP) queue
  - per-chunk engine split: scalar casts 2 w rows + evicts 2 psum rows,
    vector casts 2 + evicts 2 + does the 4 wT PSUM->SBUF copies
  - host concatenates shards, drops padding, transposes, casts to f32
"""

import math

import numpy as np

B = 512
D = 512
C = 84281
NCORES = 8
Q = 4                  # classes packed per partition line
TILE = 128 * Q         # 512 classes per tile
NT = 21                # tiles per core
CS = NT * TILE         # 10752 padded classes per core
REAL = [10536] * 7 + [C - 10536 * 7]   # real class rows per core
BASE = [10536 * i for i in range(NCORES)]
PF = 2                 # w-DMA prefetch depth in tiles

S_SCALE = 32.0
MARGIN = 0.5
COS_M = math.cos(MARGIN)
SIN_M = math.sin(MARGIN)
TH = math.cos(math.pi - MARGIN)
MM = math.sin(math.pi - MARGIN) * MARGIN

_CACHE = {}


def _build_nc():
    import concourse.tile as tile
    from concourse import bacc, mybir
    from concourse.masks import make_identity
    from contextlib import ExitStack

    f32 = mybir.dt.float32
    bf16 = mybir.dt.bfloat16

    nc = bacc.Bacc("TRN2", target_bir_lowering=False, debug=False, num_devices=NCORES)
    w_ext = nc.declare_dram_parameter("weight", [CS, D], f32, isOutput=False)
    xt_ext = nc.declare_dram_parameter("xt", [D, B], bf16, isOutput=False)
    winv_ext = nc.declare_dram_parameter("winv", [CS], f32, isOutput=False)
    out_ext = nc.declare_dram_parameter("out", [CS, B], bf16, isOutput=True)

    # class g = t*TILE + p*Q + q  ->  partition p, tile t, row q
    w_view = w_ext[:].rearrange("(t p q) d -> p t q d", p=128, q=Q)
    xt_view = xt_ext[:].rearrange("(k p) b -> p k b", p=128)        # [128, 4, B]
    winv_view = winv_ext[:].rearrange("(t p q) -> p t q", p=128, q=Q)
    out_view = out_ext[:].rearrange("(t p q) b -> p t q b", p=128, q=Q)

    with tile.TileContext(nc) as tc, ExitStack() as es:
        cpool = es.enter_context(tc.tile_pool(name="consts", bufs=1))
        wpool = es.enter_context(tc.tile_pool(name="wch", bufs=4))
        nbpool = es.enter_context(tc.tile_pool(name="wnb", bufs=3))
        outpool = es.enter_context(tc.tile_pool(name="outch", bufs=3))
        wtpool = es.enter_context(tc.tile_pool(name="wt", bufs=3))
        ppool_out = es.enter_context(tc.tile_pool(name="pout", bufs=3, space="PSUM"))
        ppool_wt = es.enter_context(tc.tile_pool(name="pwt", bufs=2, space="PSUM"))

        # ---- w prefetch: rows q0-2 on the scalar(Act) HWDGE queue, row q3
        # on the sync(SP) queue -> both queues share the 22MB input stream
        wch_tiles = []

        def issue_w_dma(t):
            wch = wpool.tile([128, Q, D], f32, tag="wch", name="wch")
            nc.scalar.dma_start(out=wch[:, 0:3, :], in_=w_view[:, t, 0:3, :])
            nc.sync.dma_start(out=wch[:, 3, :], in_=w_view[:, t, 3, :])
            wch_tiles.append(wch)

        for t in range(PF):
            issue_w_dma(t)

        ident = cpool.tile([128, 128], f32, tag="ident")
        make_identity(nc, ident[:])
        ident_bf = cpool.tile([128, 128], bf16, tag="ident_bf")
        nc.vector.tensor_copy(ident_bf[:], ident[:])

        # ---- one-shot loads: xt (pre-normalized, pre-scaled, bf16) + winv
        xnT = cpool.tile([128, 4, B], bf16, tag="xnT")
        nc.sync.dma_start(out=xnT[:], in_=xt_view)
        winv_sb = cpool.tile([128, NT, Q], f32, tag="winv_sb")
        nc.sync.dma_start(out=winv_sb[:], in_=winv_view)

        def prep(t):
            """cast w rows to bf16 (scalar q0, vector q1, gpsimd q2/q3)."""
            if t + PF < NT:
                issue_w_dma(t + PF)
            wch = wch_tiles[t]
            wnb = nbpool.tile([128, Q, D], bf16, tag="wnb", name="wnb")
            for q in range(2):
                nc.scalar.activation(
                    out=wnb[:, q, :],
                    in_=wch[:, q, :],
                    func=mybir.ActivationFunctionType.Copy,
                )
            for q in range(2, Q):
                nc.vector.tensor_copy(wnb[:, q, :], wch[:, q, :])
            return wnb

        def pe(t, wnb):
            pos = []
            for g0 in (0, 2):
                po = ppool_out.tile([128, 2 * B], f32, name="po")
                for jj in range(2):
                    j = g0 + jj
                    pwt = ppool_wt.tile([128, D], bf16, name="pwt")
                    for k in range(4):
                        nc.tensor.transpose(
                            pwt[:, k * 128 : (k + 1) * 128],
                            wnb[:, j, k * 128 : (k + 1) * 128],
                            ident_bf[:],
                        )
                    wT = wtpool.tile([128, D], bf16, tag="wT", name="wT")
                    nc.vector.tensor_copy(wT[:], pwt[:])
                    for k in range(4):
                        nc.tensor.matmul(
                            po[:, jj * B : (jj + 1) * B],
                            lhsT=wT[:, k * 128 : (k + 1) * 128],
                            rhs=xnT[:, k, :],
                            start=(k == 0),
                            stop=(k == 3),
                        )
                pos.append(po)
            return pos

        def outcopy(t, pos):
            """PSUM -> SBUF eviction with winv[c] fold (scalar q0/q1,
            vector q2/q3), then SP-queue store."""
            outch = outpool.tile([128, Q, B], bf16, tag="outch", name="outch")
            for q in range(Q):
                po = pos[q // 2]
                src = po[:, (q % 2) * B : (q % 2 + 1) * B]
                wv = winv_sb[:, t, q : q + 1]
                if q < 2:
                    nc.scalar.activation(
                        out=outch[:, q, :],
                        in_=src,
                        func=mybir.ActivationFunctionType.Copy,
                        scale=wv,
                    )
                else:
                    nc.vector.tensor_scalar_mul(outch[:, q, :], src, wv)
            nc.sync.dma_start(out=out_view[:, t, :, :], in_=outch[:])

        wnb_prev = prep(0)
        pos_prev = None
        for t in range(NT):
            if pos_prev is not None:
                outcopy(t - 1, pos_prev)
            pos = pe(t, wnb_prev)
            if t + 1 < NT:
                wnb_prev = prep(t + 1)
            pos_prev = pos
        outcopy(NT - 1, pos_prev)

    nc.finalize()
    return nc


def _get_nc():
    if "nc" not in _CACHE:
        _CACHE["nc"] = _build_nc()
    return _CACHE["nc"]


def make_in_maps(x, weight, label):
    import ml_dtypes

    x = np.asarray(x, dtype=np.float32)
    weight = np.asarray(weight, dtype=np.float32)
    xn = x / np.maximum(
        np.linalg.norm(x, axis=1, keepdims=True), 1e-12
    )
    xt = np.ascontiguousarray((S_SCALE * xn).T).astype(ml_dtypes.bfloat16)
    in_maps = []
    for i in range(NCORES):
        a, r = BASE[i], REAL[i]
        wshard = np.ones((CS, D), dtype=np.float32)
        wshard[:r] = weight[a : a + r]
        wn = np.maximum(np.sqrt(np.einsum("cd,cd->c", wshard, wshard)), 1e-12)
        winv = (1.0 / wn).astype(np.float32)
        in_maps.append({"weight": wshard, "xt": xt, "winv": winv})
    return in_maps


def assemble(results, label):
    shards = [np.asarray(results[i]["out"])[: REAL[i]] for i in range(NCORES)]
    full_t = np.concatenate(shards, axis=0).astype(np.float32)  # [C, B]
    out = np.ascontiguousarray(full_t.T)                        # [B, C]
    # margin epilogue on the 512 label positions
    label = np.asarray(label).astype(np.int64)
    b = np.arange(B)
    cosv = out[b, label] / S_SCALE
    sine = np.sqrt(np.maximum(0.0, 1.0 - cosv * cosv))
    phi = cosv * COS_M - sine * SIN_M
    out[b, label] = np.where(cosv - TH > 0, phi, cosv - MM) * S_SCALE
    return out


def kernel(x, weight, label):
    from concourse.bass_utils import run_bass_kernel_spmd

    nc = _get_nc()
    in_maps = make_in_maps(x, weight, label)
    res = run_bass_kernel_spmd(nc, in_maps, list(range(NCORES)))
    return assemble(res.results, label)


# revision 14
# speedup vs baseline: 1.2162x; 1.1318x over previous
"""ArcMarginProduct (ArcFace) forward on 8 TRN2 NeuronCores.

out[b, c] = s * cos(theta_bc)         except at c == label[b] where
out[b, c] = s * phi(cos(theta_bc))    (margin epilogue)

Strategy (classification-parallel / Partial-FC), v4:
  - pad C 84281 -> 86016 = 8 * 10752, shard class rows across 8 cores;
    Q=4 classes per partition line -> 8KB w-load / 4KB out-store
    descriptors per partition
  - host precomputes xt = bf16((s * x / ||x||).T) and winv = 1/||w_c||;
    device computes out^T[c, b] = (w_bf16 @ xt) * winv[c] via PE
    transposes + matmuls, with the per-class scale folded into the
    PSUM->SBUF eviction
  - margin epilogue (512 scattered elements) applied on host
  - w-load DMAs ride the scalar(Act) HW DGE queue (prefetch depth 2),
    out-store DMAs the sync(SP) queue
  - per-chunk engine split: scalar casts 2 w rows + evicts 2 psum rows,
    vector casts 2 + evicts 2 + does the 4 wT PSUM->SBUF copies
  - host concatenates shards, drops padding, transposes, casts to f32
"""

import math

import numpy as np

B = 512
D = 512
C = 84281
NCORES = 8
Q = 4                  # classes packed per partition line
TILE = 128 * Q         # 512 classes per tile
NT = 21                # tiles per core
CS = NT * TILE         # 10752 padded classes per core
REAL = [10536] * 7 + [C - 10536 * 7]   # real class rows per core
BASE = [10536 * i for i in range(NCORES)]
PF = 2                 # w-DMA prefetch depth in tiles

S_SCALE = 32.0
MARGIN = 0.5
COS_M = math.cos(MARGIN)
SIN_M = math.sin(MARGIN)
TH = math.cos(math.pi - MARGIN)
MM = math.sin(math.pi - MARGIN) * MARGIN

_CACHE = {}


def _build_nc():
    import concourse.tile as tile
    from concourse import bacc, mybir
    from concourse.masks import make_identity
    from contextlib import ExitStack

    f32 = mybir.dt.float32
    bf16 = mybir.dt.bfloat16

    nc = bacc.Bacc("TRN2", target_bir_lowering=False, debug=False, num_devices=NCORES)
    w_ext = nc.declare_dram_parameter("weight", [CS, D], f32, isOutput=False)
    xt_ext = nc.declare_dram_parameter("xt", [D, B], bf16, isOutput=False)
    winv_ext = nc.declare_dram_parameter("winv", [CS], f32, isOutput=False)
    out_ext = nc.declare_dram_parameter("out", [CS, B], bf16, isOutput=True)

    # class g = t*TILE + p*Q + q  ->  partition p, tile t, row q
    w_view = w_ext[:].rearrange("(t p q) d -> p t q d", p=128, q=Q)
    xt_view = xt_ext[:].rearrange("(k p) b -> p k b", p=128)        # [128, 4, B]
    winv_view = winv_ext[:].rearrange("(t p q) -> p t q", p=128, q=Q)
    out_view = out_ext[:].rearrange("(t p q) b -> p t q b", p=128, q=Q)

    with tile.TileContext(nc) as tc, ExitStack() as es:
        cpool = es.enter_context(tc.tile_pool(name="consts", bufs=1))
        wpool = es.enter_context(tc.tile_pool(name="wch", bufs=4))
        nbpool = es.enter_context(tc.tile_pool(name="wnb", bufs=3))
        outpool = es.enter_context(tc.tile_pool(name="outch", bufs=3))
        wtpool = es.enter_context(tc.tile_pool(name="wt", bufs=6))
        ppool_out = es.enter_context(tc.tile_pool(name="pout", bufs=2, space="PSUM"))
        ppool_wt = es.enter_context(tc.tile_pool(name="pwt", bufs=4, space="PSUM"))

        # ---- w prefetch: rows q0-2 on the scalar(Act) HWDGE queue, row q3
        # on the sync(SP) queue -> both queues share the 22MB input stream
        wch_tiles = []

        def issue_w_dma(t):
            wch = wpool.tile([128, Q, D], f32, tag="wch", name="wch")
            nc.scalar.dma_start(out=wch[:, 0:3, :], in_=w_view[:, t, 0:3, :])
            nc.sync.dma_start(out=wch[:, 3, :], in_=w_view[:, t, 3, :])
            wch_tiles.append(wch)

        for t in range(PF):
            issue_w_dma(t)

        ident = cpool.tile([128, 128], f32, tag="ident")
        make_identity(nc, ident[:])
        ident_bf = cpool.tile([128, 128], bf16, tag="ident_bf")
        nc.vector.tensor_copy(ident_bf[:], ident[:])

        # ---- one-shot loads: xt (pre-normalized, pre-scaled, bf16) + winv
        xnT = cpool.tile([128, 4, B], bf16, tag="xnT")
        nc.sync.dma_start(out=xnT[:], in_=xt_view)
        winv_sb = cpool.tile([128, NT, Q], f32, tag="winv_sb")
        nc.sync.dma_start(out=winv_sb[:], in_=winv_view)

        def prep(t):
            """cast w rows to bf16 (scalar q0, vector q1, gpsimd q2/q3)."""
            if t + PF < NT:
                issue_w_dma(t + PF)
            wch = wch_tiles[t]
            wnb = nbpool.tile([128, Q, D], bf16, tag="wnb", name="wnb")
            for q in range(2):
                nc.scalar.activation(
                    out=wnb[:, q, :],
                    in_=wch[:, q, :],
                    func=mybir.ActivationFunctionType.Copy,
                )
            for q in range(2, Q):
                nc.vector.tensor_copy(wnb[:, q, :], wch[:, q, :])
            return wnb

        def pe(t, wnb):
            # all 16 transposes first (one long PE streak), wT copies
            # pipeline underneath, then all 16 matmuls (second streak) --
            # long unbroken PE bursts keep the tensor engine at max p-state
            wts = []
            for j in range(Q):
                pwt = ppool_wt.tile([128, D], bf16, name="pwt")
                for k in range(4):
                    nc.tensor.transpose(
                        pwt[:, k * 128 : (k + 1) * 128],
                        wnb[:, j, k * 128 : (k + 1) * 128],
                        ident_bf[:],
                    )
                wT = wtpool.tile([128, D], bf16, tag="wT", name="wT")
                nc.vector.tensor_copy(wT[:], pwt[:])
                wts.append(wT)
            pos = []
            for g0 in (0, 2):
                po = ppool_out.tile([128, 2 * B], f32, name="po")
                for jj in range(2):
                    j = g0 + jj
                    for k in range(4):
                        nc.tensor.matmul(
                            po[:, jj * B : (jj + 1) * B],
                            lhsT=wts[j][:, k * 128 : (k + 1) * 128],
                            rhs=xnT[:, k, :],
                            start=(k == 0),
                            stop=(k == 3),
                        )
                pos.append(po)
            return pos

        def outcopy(t, pos):
            """PSUM -> SBUF eviction with winv[c] fold (scalar q0/q1,
            vector q2/q3), then SP-queue store."""
            outch = outpool.tile([128, Q, B], bf16, tag="outch", name="outch")
            for q in range(Q):
                po = pos[q // 2]
                src = po[:, (q % 2) * B : (q % 2 + 1) * B]
                wv = winv_sb[:, t, q : q + 1]
                if q < 2:
                    nc.scalar.activation(
                        out=outch[:, q, :],
                        in_=src,
                        func=mybir.ActivationFunctionType.Copy,
                        scale=wv,
                    )
                else:
                    nc.vector.tensor_scalar_mul(outch[:, q, :], src, wv)
            nc.sync.dma_start(out=out_view[:, t, :, :], in_=outch[:])

        wnb_prev = prep(0)
        pos_prev = None
        for t in range(NT):
            if pos_prev is not None:
                outcopy(t - 1, pos_prev)
            pos = pe(t, wnb_prev)
            if t + 1 < NT:
                wnb_prev = prep(t + 1)
            pos_prev = pos
        outcopy(NT - 1, pos_prev)

    nc.finalize()
    return nc


def _get_nc():
    if "nc" not in _CACHE:
        _CACHE["nc"] = _build_nc()
    return _CACHE["nc"]


def make_in_maps(x, weight, label):
    import ml_dtypes

    x = np.asarray(x, dtype=np.float32)
    weight = np.asarray(weight, dtype=np.float32)
    xn = x / np.maximum(
        np.linalg.norm(x, axis=1, keepdims=True), 1e-12
    )
    xt = np.ascontiguousarray((S_SCALE * xn).T).astype(ml_dtypes.bfloat16)
    in_maps = []
    for i in range(NCORES):
        a, r = BASE[i], REAL[i]
        wshard = np.ones((CS, D), dtype=np.float32)
        wshard[:r] = weight[a : a + r]
        wn = np.maximum(np.sqrt(np.einsum("cd,cd->c", wshard, wshard)), 1e-12)
        winv = (1.0 / wn).astype(np.float32)
        in_maps.append({"weight": wshard, "xt": xt, "winv": winv})
    return in_maps


def assemble(results, label):
    shards = [np.asarray(results[i]["out"])[: REAL[i]] for i in range(NCORES)]
    full_t = np.concatenate(shards, axis=0).astype(np.float32)  # [C, B]
    out = np.ascontiguousarray(full_t.T)                        # [B, C]
    # margin epilogue on the 512 label positions
    label = np.asarray(label).astype(np.int64)
    b = np.arange(B)
    cosv = out[b, label] / S_SCALE
    sine = np.sqrt(np.maximum(0.0, 1.0 - cosv * cosv))
    phi = cosv * COS_M - sine * SIN_M
    out[b, label] = np.where(cosv - TH > 0, phi, cosv - MM) * S_SCALE
    return out


def kernel(x, weight, label):
    from concourse.bass_utils import run_bass_kernel_spmd

    nc = _get_nc()
    in_maps = make_in_maps(x, weight, label)
    res = run_bass_kernel_spmd(nc, in_maps, list(range(NCORES)))
    return assemble(res.results, label)


# revision 15
# speedup vs baseline: 1.6574x; 1.3628x over previous
"""ArcMarginProduct (ArcFace) forward on 8 TRN2 NeuronCores.

out[b, c] = s * cos(theta_bc)         except at c == label[b] where
out[b, c] = s * phi(cos(theta_bc))    (margin epilogue)

Strategy (classification-parallel / Partial-FC), v8:
  - pad C 84281 -> 86016 = 8 * 10752, shard class rows across 8 cores
  - host precomputes xt = bf16((s * x / ||x||).T)  [D, B] and
    wnT = bf16((w / ||w_c||).T)  [D, CS] per shard -> the device kernel
    is a pure bf16 matmul: out^T[c, b] = wnT^T @ xt, PE-bound, with no
    on-device transposes, casts, or normalization
  - margin epilogue (512 scattered label positions) applied on host
  - wnT-load DMAs ride the scalar(Act) HW DGE queue (prefetch depth 2),
    out-store DMAs the sync(SP) queue -> independent streams
  - PSUM->SBUF eviction split: scalar takes 2 of 4 class windows
    (activation Copy), vector the other 2 (tensor_copy)
  - host concatenates shards, drops padding, transposes, casts to f32
"""

import math

import numpy as np

B = 512
D = 512
C = 84281
NCORES = 8
TILE = 512             # classes per tile (4 matmul M-windows of 128)
NT = 21                # tiles per core
CS = NT * TILE         # 10752 padded classes per core
REAL = [10536] * 7 + [C - 10536 * 7]   # real class rows per core
BASE = [10536 * i for i in range(NCORES)]
PF = 2                 # w-DMA prefetch depth in tiles

S_SCALE = 32.0
MARGIN = 0.5
COS_M = math.cos(MARGIN)
SIN_M = math.sin(MARGIN)
TH = math.cos(math.pi - MARGIN)
MM = math.sin(math.pi - MARGIN) * MARGIN

_CACHE = {}


def _build_nc():
    import concourse.tile as tile
    from concourse import bacc, mybir
    from contextlib import ExitStack

    bf16 = mybir.dt.bfloat16
    f32 = mybir.dt.float32

    nc = bacc.Bacc("TRN2", target_bir_lowering=False, debug=False, num_devices=NCORES)
    w_ext = nc.declare_dram_parameter("wnt", [D, CS], bf16, isOutput=False)
    xt_ext = nc.declare_dram_parameter("xt", [D, B], bf16, isOutput=False)
    out_ext = nc.declare_dram_parameter("out", [CS, B], bf16, isOutput=True)

    # class g = t*TILE + m*128 + p  ->  tile t, M-window m, psum partition p
    w_view = w_ext[:].rearrange("(k p) c -> p k c", p=128)          # [128, 4, CS]
    xt_view = xt_ext[:].rearrange("(k p) b -> p k b", p=128)        # [128, 4, B]
    out_view = out_ext[:].rearrange("(t m p) b -> p t m b", p=128, m=4)

    with tile.TileContext(nc) as tc, ExitStack() as es:
        cpool = es.enter_context(tc.tile_pool(name="consts", bufs=1))
        wpool = es.enter_context(tc.tile_pool(name="wch", bufs=4))
        outpool = es.enter_context(tc.tile_pool(name="outch", bufs=3))
        ppool_out = es.enter_context(tc.tile_pool(name="pout", bufs=4, space="PSUM"))

        # ---- w prefetch (scalar HWDGE queue)
        wch_tiles = []

        def issue_w_dma(t):
            wch = wpool.tile([128, 4, TILE], bf16, tag="wch", name="wch")
            nc.scalar.dma_start(out=wch[:], in_=w_view[:, :, t * TILE : (t + 1) * TILE])
            wch_tiles.append(wch)

        for t in range(PF):
            issue_w_dma(t)

        # ---- one-shot load: xt (pre-normalized, pre-scaled, bf16)
        xnT = cpool.tile([128, 4, B], bf16, tag="xnT")
        nc.sync.dma_start(out=xnT[:], in_=xt_view)

        def pe(t):
            wch = wch_tiles[t]
            if t + PF < NT:
                issue_w_dma(t + PF)
            pos = []
            for g0 in (0, 2):
                po = ppool_out.tile([128, 2 * B], f32, name="po")
                for jj in range(2):
                    m = g0 + jj
                    for k in range(4):
                        nc.tensor.matmul(
                            po[:, jj * B : (jj + 1) * B],
                            lhsT=wch[:, k, m * 128 : (m + 1) * 128],
                            rhs=xnT[:, k, :],
                            start=(k == 0),
                            stop=(k == 3),
                        )
                pos.append(po)
            return pos

        def outcopy(t, pos):
            """PSUM -> SBUF eviction (scalar m0/m1, vector m2/m3), SP store."""
            outch = outpool.tile([128, 4, B], bf16, tag="outch", name="outch")
            for m in range(4):
                src = pos[m // 2][:, (m % 2) * B : (m % 2 + 1) * B]
                if m < 2:
                    nc.scalar.activation(
                        out=outch[:, m, :],
                        in_=src,
                        func=mybir.ActivationFunctionType.Copy,
                    )
                else:
                    nc.vector.tensor_copy(outch[:, m, :], src)
            nc.sync.dma_start(out=out_view[:, t, :, :], in_=outch[:])

        pos_prev = None
        for t in range(NT):
            if pos_prev is not None:
                outcopy(t - 1, pos_prev)
            pos = pe(t)
            pos_prev = pos
        outcopy(NT - 1, pos_prev)

    nc.finalize()
    return nc


def _get_nc():
    if "nc" not in _CACHE:
        _CACHE["nc"] = _build_nc()
    return _CACHE["nc"]


def make_in_maps(x, weight, label):
    import ml_dtypes

    x = np.asarray(x, dtype=np.float32)
    weight = np.asarray(weight, dtype=np.float32)
    xn = x / np.maximum(np.linalg.norm(x, axis=1, keepdims=True), 1e-12)
    xt = np.ascontiguousarray((S_SCALE * xn).T).astype(ml_dtypes.bfloat16)
    wn = weight / np.maximum(
        np.sqrt(np.einsum("cd,cd->c", weight, weight))[:, None], 1e-12
    )
    in_maps = []
    for i in range(NCORES):
        a, r = BASE[i], REAL[i]
        wshard = np.empty((CS, D), dtype=np.float32)
        wshard[:r] = wn[a : a + r]
        wshard[r:] = 1.0
        wnt = np.ascontiguousarray(wshard.T).astype(ml_dtypes.bfloat16)
        in_maps.append({"wnt": wnt, "xt": xt})
    return in_maps


def assemble(results, label):
    shards = [np.asarray(results[i]["out"])[: REAL[i]] for i in range(NCORES)]
    full_t = np.concatenate(shards, axis=0).astype(np.float32)  # [C, B]
    out = np.ascontiguousarray(full_t.T)                        # [B, C]
    # margin epilogue on the 512 label positions
    label = np.asarray(label).astype(np.int64)
    b = np.arange(B)
    cosv = out[b, label] / S_SCALE
    sine = np.sqrt(np.maximum(0.0, 1.0 - cosv * cosv))
    phi = cosv * COS_M - sine * SIN_M
    out[b, label] = np.where(cosv - TH > 0, phi, cosv - MM) * S_SCALE
    return out


def kernel(x, weight, label):
    from concourse.bass_utils import run_bass_kernel_spmd

    nc = _get_nc()
    in_maps = make_in_maps(x, weight, label)
    res = run_bass_kernel_spmd(nc, in_maps, list(range(NCORES)))
    return assemble(res.results, label)


# revision 16
# speedup vs baseline: 1.7068x; 1.0298x over previous
"""ArcMarginProduct (ArcFace) forward on 8 TRN2 NeuronCores.

out[b, c] = s * cos(theta_bc)         except at c == label[b] where
out[b, c] = s * phi(cos(theta_bc))    (margin epilogue)

Strategy (classification-parallel / Partial-FC), v8:
  - pad C 84281 -> 86016 = 8 * 10752, shard class rows across 8 cores
  - host precomputes xt = bf16((s * x / ||x||).T)  [D, B] and
    wnT = bf16((w / ||w_c||).T)  [D, CS] per shard -> the device kernel
    is a pure bf16 matmul: out^T[c, b] = wnT^T @ xt, PE-bound, with no
    on-device transposes, casts, or normalization
  - margin epilogue (512 scattered label positions) applied on host
  - wnT-load DMAs ride the scalar(Act) HW DGE queue (prefetch depth 2),
    out-store DMAs the sync(SP) queue -> independent streams
  - PSUM->SBUF eviction split: scalar takes 2 of 4 class windows
    (activation Copy), vector the other 2 (tensor_copy)
  - host concatenates shards, drops padding, transposes, casts to f32
"""

import math

import numpy as np

B = 512
D = 512
C = 84281
NCORES = 8
TILE = 512             # classes per tile (4 matmul M-windows of 128)
NT = 21                # tiles per core
CS = NT * TILE         # 10752 padded classes per core
REAL = [10536] * 7 + [C - 10536 * 7]   # real class rows per core
BASE = [10536 * i for i in range(NCORES)]
PF = 2                 # w-DMA prefetch depth in tiles

S_SCALE = 32.0
MARGIN = 0.5
COS_M = math.cos(MARGIN)
SIN_M = math.sin(MARGIN)
TH = math.cos(math.pi - MARGIN)
MM = math.sin(math.pi - MARGIN) * MARGIN

_CACHE = {}


def _build_nc():
    import concourse.tile as tile
    from concourse import bacc, mybir
    from contextlib import ExitStack

    bf16 = mybir.dt.bfloat16
    f32 = mybir.dt.float32

    nc = bacc.Bacc("TRN2", target_bir_lowering=False, debug=False, num_devices=NCORES)
    w_ext = nc.declare_dram_parameter("wnt", [D, CS], bf16, isOutput=False)
    xt_ext = nc.declare_dram_parameter("xt", [D, B], bf16, isOutput=False)
    out_ext = nc.declare_dram_parameter("out", [CS, B], bf16, isOutput=True)

    # class g = t*TILE + m*128 + p  ->  tile t, M-window m, psum partition p
    w_view = w_ext[:].rearrange("(k p) c -> p k c", p=128)          # [128, 4, CS]
    xt_view = xt_ext[:].rearrange("(k p) b -> p k b", p=128)        # [128, 4, B]
    out_view = out_ext[:].rearrange("(t m p) b -> p t m b", p=128, m=4)

    with tile.TileContext(nc) as tc, ExitStack() as es:
        cpool = es.enter_context(tc.tile_pool(name="consts", bufs=1))
        wpool = es.enter_context(tc.tile_pool(name="wch", bufs=4))
        outpool = es.enter_context(tc.tile_pool(name="outch", bufs=3))
        ppool_out = es.enter_context(tc.tile_pool(name="pout", bufs=4, space="PSUM"))

        # ---- w prefetch (scalar HWDGE queue)
        wch_tiles = []

        def issue_w_dma(t):
            wch = wpool.tile([128, 4, TILE], bf16, tag="wch", name="wch")
            if t == 0:
                # split the first load so the very first matmul window can
                # start as soon as its 0.13MB lands
                nc.scalar.dma_start(
                    out=wch[:, :, 0:128], in_=w_view[:, :, 0:128]
                )
                nc.scalar.dma_start(
                    out=wch[:, :, 128:TILE], in_=w_view[:, :, 128:TILE]
                )
            else:
                nc.scalar.dma_start(
                    out=wch[:], in_=w_view[:, :, t * TILE : (t + 1) * TILE]
                )
            wch_tiles.append(wch)

        # ---- one-shot load: xt (pre-normalized, pre-scaled, bf16),
        # k=0 slice first so the first k-accumulation can begin early
        xnT = cpool.tile([128, 4, B], bf16, tag="xnT")
        nc.sync.dma_start(out=xnT[:, 0, :], in_=xt_view[:, 0, :])
        nc.sync.dma_start(out=xnT[:, 1:4, :], in_=xt_view[:, 1:4, :])

        for t in range(PF):
            issue_w_dma(t)

        def pe(t):
            wch = wch_tiles[t]
            if t + PF < NT:
                issue_w_dma(t + PF)
            pos = []
            for g0 in (0, 2):
                po = ppool_out.tile([128, 2 * B], f32, name="po")
                for jj in range(2):
                    m = g0 + jj
                    for k in range(4):
                        nc.tensor.matmul(
                            po[:, jj * B : (jj + 1) * B],
                            lhsT=wch[:, k, m * 128 : (m + 1) * 128],
                            rhs=xnT[:, k, :],
                            start=(k == 0),
                            stop=(k == 3),
                        )
                pos.append(po)
            return pos

        def outcopy(t, pos):
            """PSUM -> SBUF eviction (scalar m0/m1, vector m2/m3), SP store."""
            outch = outpool.tile([128, 4, B], bf16, tag="outch", name="outch")
            for m in range(4):
                src = pos[m // 2][:, (m % 2) * B : (m % 2 + 1) * B]
                if m < 2:
                    nc.scalar.activation(
                        out=outch[:, m, :],
                        in_=src,
                        func=mybir.ActivationFunctionType.Copy,
                    )
                else:
                    nc.vector.tensor_copy(outch[:, m, :], src)
            nc.sync.dma_start(out=out_view[:, t, :, :], in_=outch[:])

        pos_prev = None
        for t in range(NT):
            if pos_prev is not None:
                outcopy(t - 1, pos_prev)
            pos = pe(t)
            pos_prev = pos
        outcopy(NT - 1, pos_prev)

    nc.finalize()
    return nc


def _get_nc():
    if "nc" not in _CACHE:
        _CACHE["nc"] = _build_nc()
    return _CACHE["nc"]


def make_in_maps(x, weight, label):
    import ml_dtypes

    x = np.asarray(x, dtype=np.float32)
    weight = np.asarray(weight, dtype=np.float32)
    xn = x / np.maximum(np.linalg.norm(x, axis=1, keepdims=True), 1e-12)
    xt = np.ascontiguousarray((S_SCALE * xn).T).astype(ml_dtypes.bfloat16)
    wn = weight / np.maximum(
        np.sqrt(np.einsum("cd,cd->c", weight, weight))[:, None], 1e-12
    )
    in_maps = []
    for i in range(NCORES):
        a, r = BASE[i], REAL[i]
        wshard = np.empty((CS, D), dtype=np.float32)
        wshard[:r] = wn[a : a + r]
        wshard[r:] = 1.0
        wnt = np.ascontiguousarray(wshard.T).astype(ml_dtypes.bfloat16)
        in_maps.append({"wnt": wnt, "xt": xt})
    return in_maps


def assemble(results, label):
    shards = [np.asarray(results[i]["out"])[: REAL[i]] for i in range(NCORES)]
    full_t = np.concatenate(shards, axis=0).astype(np.float32)  # [C, B]
    out = np.ascontiguousarray(full_t.T)                        # [B, C]
    # margin epilogue on the 512 label positions
    label = np.asarray(label).astype(np.int64)
    b = np.arange(B)
    cosv = out[b, label] / S_SCALE
    sine = np.sqrt(np.maximum(0.0, 1.0 - cosv * cosv))
    phi = cosv * COS_M - sine * SIN_M
    out[b, label] = np.where(cosv - TH > 0, phi, cosv - MM) * S_SCALE
    return out


def kernel(x, weight, label):
    from concourse.bass_utils import run_bass_kernel_spmd

    nc = _get_nc()
    in_maps = make_in_maps(x, weight, label)
    res = run_bass_kernel_spmd(nc, in_maps, list(range(NCORES)))
    return assemble(res.results, label)


# revision 19
# speedup vs baseline: 1.7140x; 1.0042x over previous
"""ArcMarginProduct (ArcFace) forward on 8 TRN2 NeuronCores.

out[b, c] = s * cos(theta_bc)         except at c == label[b] where
out[b, c] = s * phi(cos(theta_bc))    (margin epilogue)

Strategy (classification-parallel / Partial-FC), v8:
  - pad C 84281 -> 86016 = 8 * 10752, shard class rows across 8 cores
  - host precomputes xt = bf16((s * x / ||x||).T)  [D, B] and
    wnT = bf16((w / ||w_c||).T)  [D, CS] per shard -> the device kernel
    is a pure bf16 matmul: out^T[c, b] = wnT^T @ xt, PE-bound, with no
    on-device transposes, casts, or normalization
  - margin epilogue (512 scattered label positions) applied on host
  - wnT-load DMAs ride the scalar(Act) HW DGE queue (prefetch depth 2),
    out-store DMAs the sync(SP) queue -> independent streams
  - PSUM->SBUF eviction split: scalar takes 2 of 4 class windows
    (activation Copy), vector the other 2 (tensor_copy)
  - host concatenates shards, drops padding, transposes, casts to f32
"""

import math

import numpy as np

B = 512
D = 512
C = 84281
NCORES = 8
TILE = 512             # classes per tile (4 matmul M-windows of 128)
NT = 21                # tiles per core
CS = NT * TILE         # 10752 padded classes per core
REAL = [10536] * 7 + [C - 10536 * 7]   # real class rows per core
BASE = [10536 * i for i in range(NCORES)]
PF = 2                 # w-DMA prefetch depth in tiles

S_SCALE = 32.0
MARGIN = 0.5
COS_M = math.cos(MARGIN)
SIN_M = math.sin(MARGIN)
TH = math.cos(math.pi - MARGIN)
MM = math.sin(math.pi - MARGIN) * MARGIN

_CACHE = {}


def _build_nc():
    import concourse.tile as tile
    from concourse import bacc, mybir
    from contextlib import ExitStack

    bf16 = mybir.dt.bfloat16
    f32 = mybir.dt.float32

    nc = bacc.Bacc("TRN2", target_bir_lowering=False, debug=False, num_devices=NCORES)
    w_ext = nc.declare_dram_parameter("wnt", [D, CS], bf16, isOutput=False)
    xt_ext = nc.declare_dram_parameter("xt", [D, B], bf16, isOutput=False)
    out_ext = nc.declare_dram_parameter("out", [CS, B], bf16, isOutput=True)

    # class g = t*TILE + m*128 + p  ->  tile t, M-window m, psum partition p
    w_view = w_ext[:].rearrange("(k p) c -> p k c", p=128)          # [128, 4, CS]
    xt_view = xt_ext[:].rearrange("(k p) b -> p k b", p=128)        # [128, 4, B]
    out_view = out_ext[:].rearrange("(t m p) b -> p t m b", p=128, m=4)

    with tile.TileContext(nc) as tc, ExitStack() as es:
        cpool = es.enter_context(tc.tile_pool(name="consts", bufs=1))
        wpool = es.enter_context(tc.tile_pool(name="wch", bufs=4))
        outpool = es.enter_context(tc.tile_pool(name="outch", bufs=3))
        ppool_out = es.enter_context(tc.tile_pool(name="pout", bufs=4, space="PSUM"))

        # ---- w prefetch (scalar HWDGE queue)
        wch_tiles = []

        def issue_w_dma(t):
            wch = wpool.tile([128, 4, TILE], bf16, tag="wch", name="wch")
            if t == 0:
                # split the first load so the very first matmul window can
                # start as soon as its 0.13MB lands
                nc.scalar.dma_start(
                    out=wch[:, :, 0:128], in_=w_view[:, :, 0:128]
                )
                nc.scalar.dma_start(
                    out=wch[:, :, 128:TILE], in_=w_view[:, :, 128:TILE]
                )
            else:
                nc.scalar.dma_start(
                    out=wch[:], in_=w_view[:, :, t * TILE : (t + 1) * TILE]
                )
            wch_tiles.append(wch)

        # ---- one-shot load: xt (pre-normalized, pre-scaled, bf16),
        # one DMA per k slice so each k-accumulation can begin early
        xnT = cpool.tile([128, 4, B], bf16, tag="xnT")
        for k in range(4):
            nc.sync.dma_start(out=xnT[:, k, :], in_=xt_view[:, k, :])

        for t in range(PF):
            issue_w_dma(t)

        def pe(t):
            wch = wch_tiles[t]
            if t + PF < NT:
                issue_w_dma(t + PF)
            pos = []
            for g0 in (0, 2):
                po = ppool_out.tile([128, 2 * B], f32, name="po")
                for jj in range(2):
                    m = g0 + jj
                    for k in range(4):
                        nc.tensor.matmul(
                            po[:, jj * B : (jj + 1) * B],
                            lhsT=wch[:, k, m * 128 : (m + 1) * 128],
                            rhs=xnT[:, k, :],
                            start=(k == 0),
                            stop=(k == 3),
                        )
                pos.append(po)
            return pos

        def outcopy(t, pos, split_store=False):
            """PSUM -> SBUF eviction (scalar m0/m1, vector m2/m3), SP store."""
            outch = outpool.tile([128, 4, B], bf16, tag="outch", name="outch")
            for m in range(4):
                src = pos[m // 2][:, (m % 2) * B : (m % 2 + 1) * B]
                if m < 2:
                    nc.scalar.activation(
                        out=outch[:, m, :],
                        in_=src,
                        func=mybir.ActivationFunctionType.Copy,
                    )
                else:
                    nc.vector.tensor_copy(outch[:, m, :], src)
            if split_store:
                nc.sync.dma_start(out=out_view[:, t, 0:2, :], in_=outch[:, 0:2, :])
                nc.sync.dma_start(out=out_view[:, t, 2:4, :], in_=outch[:, 2:4, :])
            else:
                nc.sync.dma_start(out=out_view[:, t, :, :], in_=outch[:])

        pos_prev = None
        for t in range(NT):
            if pos_prev is not None:
                outcopy(t - 1, pos_prev)
            pos = pe(t)
            pos_prev = pos
        outcopy(NT - 1, pos_prev, split_store=True)

    nc.finalize()
    return nc


def _get_nc():
    if "nc" not in _CACHE:
        _CACHE["nc"] = _build_nc()
    return _CACHE["nc"]


def make_in_maps(x, weight, label):
    import ml_dtypes

    x = np.asarray(x, dtype=np.float32)
    weight = np.asarray(weight, dtype=np.float32)
    xn = x / np.maximum(np.linalg.norm(x, axis=1, keepdims=True), 1e-12)
    xt = np.ascontiguousarray((S_SCALE * xn).T).astype(ml_dtypes.bfloat16)
    wn = weight / np.maximum(
        np.sqrt(np.einsum("cd,cd->c", weight, weight))[:, None], 1e-12
    )
    in_maps = []
    for i in range(NCORES):
        a, r = BASE[i], REAL[i]
        wshard = np.empty((CS, D), dtype=np.float32)
        wshard[:r] = wn[a : a + r]
        wshard[r:] = 1.0
        wnt = np.ascontiguousarray(wshard.T).astype(ml_dtypes.bfloat16)
        in_maps.append({"wnt": wnt, "xt": xt})
    return in_maps


def assemble(results, label):
    shards = [np.asarray(results[i]["out"])[: REAL[i]] for i in range(NCORES)]
    full_t = np.concatenate(shards, axis=0).astype(np.float32)  # [C, B]
    out = np.ascontiguousarray(full_t.T)                        # [B, C]
    # margin epilogue on the 512 label positions
    label = np.asarray(label).astype(np.int64)
    b = np.arange(B)
    cosv = out[b, label] / S_SCALE
    sine = np.sqrt(np.maximum(0.0, 1.0 - cosv * cosv))
    phi = cosv * COS_M - sine * SIN_M
    out[b, label] = np.where(cosv - TH > 0, phi, cosv - MM) * S_SCALE
    return out


def kernel(x, weight, label):
    from concourse.bass_utils import run_bass_kernel_spmd

    nc = _get_nc()
    in_maps = make_in_maps(x, weight, label)
    res = run_bass_kernel_spmd(nc, in_maps, list(range(NCORES)))
    return assemble(res.results, label)
